# revision 1
# baseline (speedup 1.0000x reference)
"""Trainium2 Bass kernel for nn_CostFn_18562848653837.

reference(x, cond, time) only reads x[b, j, 6+k] for j in [0,26), k in [0,6)
(~2.6 MB of the 436 MB input; cond/time are unused) and computes, per point,
the reflected mass 1 / (u^T J M^{-1} J^T u) with u = e_x, which reduces via
Sherman-Morrison (M = 2I + 0.5 c c^T, c = cos(cq), s = sin(cq), v = L*s,
cq = cumsum(q)) to

    denom = 0.5*||v||^2 - 0.125*(c.v)^2 / (1 + 0.25*||c||^2)

and further, with double-angle identities, to pure functions of sin^2(cq)
and sin(2*cq):

    ||v||^2 = sum_k L_k^2 sin^2(cq_k)            =: Q1
    c.v     = 0.5 * sum_k L_k sin(2 cq_k)        =  0.5 * P2
    ||c||^2 = 6 - sum_k sin^2(cq_k)              =  6 - Q3
    denom   = 0.5*Q1 - 0.03125*P2^2 / (2.5 - 0.25*Q3)

Both sin^2(th) and sin(2 th) are invariant under th -> th - k*pi for any
integer k, so range reduction mod pi needs no off-by-one fixup. The host
ships q/pi, so the device cumsum yields g = cq/pi and the reduction is a
single fused (g + 1.5*2^23) - 1.5*2^23 tensor_scalar (f32 round-to-nearest,
HW-probed to round between ALU stages) plus a subtract: m = g - rne(g),
|m| <= 0.5 exactly. The ACT Sin applies the radians conversion through its
input scale (pi and a one-ulp-shaded 2*pi), keeping its [-pi, pi] table
domain satisfied unconditionally.

Work is spread over engines: cumsum + critical-slice range reduction + WA +
Q1 + denominator chain on DVE; remaining range-reduction slices + WS + P2 +
Q3 + TC on GpSimd (Pool); the two Sins on ACT (with a dep-free warm-up Sin
so the table load hides behind the input DMAs).

Sharding: pure data parallel over batch - core i gets batches
[512*i, 512*(i+1)), i.e. 512*26 = 13312 points laid out as a (128, 104) tile
per q-component. Each core emits one f32 partial sum; host adds the 8.
"""

import numpy as np

_P, _W, _K = 128, 104, 6
_F = _K * _W
_NCORES = 8
_B, _H, _T = 4096, 1024, 26
_BPC = _B // _NCORES  # batches per core

_CACHE = {}


def _get_nc():
    if "nc" in _CACHE:
        return _CACHE["nc"]

    import concourse.tile as tile
    import concourse.mybir as mybir
    from concourse import bacc

    PI32 = float(np.float32(np.pi))
    # One-ulp-shaded 2*pi: |m| <= 0.5 exactly (RNE ties), so the Sin input
    # |SCALE2*m| <= pi*(1-2^-23) stays strictly inside the table domain.
    SCALE2 = float(np.float32(2.0 * np.pi * (1.0 - 2.0**-23)))
    MAGIC = 12582912.0  # 1.5 * 2^23: f32 add/sub rounds to nearest int
    L = [float(np.float32(v)) for v in np.arange(1, 7) * 0.1 + 0.3]

    f32 = mybir.dt.float32
    AX = mybir.AxisListType
    OP = mybir.AluOpType
    ACT = mybir.ActivationFunctionType

    # disable_frame_to_traceback keeps source paths/line numbers out of the
    # BIR so the neuronx compile cache hits regardless of where this file
    # lives (and across edits that only shift line numbers)
    nc = bacc.Bacc(
        "TRN2", target_bir_lowering=False, debug=False, num_devices=_NCORES,
        disable_frame_to_traceback=True,
    )
    q_dram = nc.dram_tensor("q", [_K, _P, _W], f32, kind="ExternalInput")
    out_dram = nc.dram_tensor("out", [_P, 1], f32, kind="ExternalOutput")

    with (
        tile.TileContext(nc) as tc,
        tc.tile_pool(name="pool", bufs=1) as pool,
    ):
        # constant bias for TC on ACT, built on DVE while DMAs are in flight
        B25 = pool.tile([_P, 1], f32)
        nc.vector.memset(B25[:], 2.5)

        # Dep-free dummy Sin on the pre-initialized const-1.0 AP: the Sin
        # table-set load is hoisted before ACT's first Sin, and by making
        # that first Sin dependency-free the ~1.3us load runs at t~0,
        # hidden behind the input DMAs instead of stalling the real Sin.
        one_ap = nc.const_aps.aps[(f32, 1.0)]
        WARM = pool.tile([_P, 1], f32)
        nc.scalar.activation(WARM[:], one_ap[:_P], ACT.Sin)

        # one tile per q-plane so the cumsum can chase the DMAs; split the
        # issues across the two DMA-capable sequencers (500 ns issue each)
        Qk = []
        for k in range(_K):
            qk = pool.tile([_P, _W], f32, tag=f"q{k}")
            eng = nc.sync if k % 2 == 0 else nc.gpsimd
            eng.dma_start(qk[:], q_dram[k])
            Qk.append(qk)

        # The host ships q/pi, so the cumsum produces g = cq/pi directly;
        # +8 is seeded into block 0 (a multiple of pi in these units, to
        # which the double-angle quantities are invariant) so g > 0.
        CQ = pool.tile([_P, _F], f32)
        nc.vector.tensor_scalar(CQ[:, 0:_W], Qk[0][:], 8.0, None, OP.add)
        for k in range(1, _K):
            nc.vector.tensor_add(
                CQ[:, k * _W : (k + 1) * _W],
                CQ[:, (k - 1) * _W : k * _W],
                Qk[k][:],
            )

        # Range reduction in pi-units: k = rne(g) via one fused
        # (g + MAGIC) - MAGIC tensor_scalar (HW-probed: both DVE and Pool
        # round to f32 between ALU stages), then m = g - k exactly, with
        # |m| <= 0.5 guaranteed. Pipelined behind the cumsum: planes 0..4
        # on Pool, the critical last plane on DVE.
        KR = pool.tile([_P, _F], f32)
        RC = pool.tile([_P, _F], f32)
        for k in range(_K):
            sl = slice(k * _W, (k + 1) * _W)
            eng = nc.vector if k == _K - 1 else nc.gpsimd
            eng.tensor_scalar(KR[:, sl], CQ[:, sl], MAGIC, MAGIC, OP.add, OP.subtract)
            eng.tensor_sub(RC[:, sl], CQ[:, sl], KR[:, sl])

        # ACT applies the radians conversion for free via its input scale:
        # sin(pi*m) = +-sin(cq), sin(2pi*m) = sin(2cq) exactly. SM is split
        # so planes 0..4 (whose range reduction lands first, on Pool) start
        # ~700ns before plane 5's DVE-side reduction completes.
        SM = pool.tile([_P, _F], f32)
        nc.scalar.activation(
            SM[:, 0 : 5 * _W], RC[:, 0 : 5 * _W], ACT.Sin, scale=PI32
        )
        nc.scalar.activation(
            SM[:, 5 * _W : _F], RC[:, 5 * _W : _F], ACT.Sin, scale=PI32
        )
        SF = pool.tile([_P, _F], f32)
        nc.scalar.activation(SF[:], RC[:], ACT.Sin, scale=SCALE2)

        # WA_k = L_k^2 sin^2 fused from SM on DVE (one stt per plane, no
        # full-width square pass); sin^2 planes + Q3 partial sums chase the
        # first SM half on Pool, which is otherwise idle until SF lands
        WA = pool.tile([_P, _F], f32)
        SMSQ = pool.tile([_P, _F], f32)
        WS = pool.tile([_P, _F], f32)
        Q1 = pool.tile([_P, _W], f32)
        P2 = pool.tile([_P, _W], f32)
        Q3 = pool.tile([_P, _W], f32)
        for k in range(_K):
            sl = slice(k * _W, (k + 1) * _W)
            nc.vector.scalar_tensor_tensor(
                WA[:, sl], SM[:, sl], L[k] * L[k], SM[:, sl], OP.mult, OP.mult
            )
        for k in range(2):
            sl = slice(k * _W, (k + 1) * _W)
            nc.gpsimd.tensor_mul(SMSQ[:, sl], SM[:, sl], SM[:, sl])
        nc.gpsimd.tensor_add(Q3[:], SMSQ[:, 0:_W], SMSQ[:, _W : 2 * _W])
        for k in range(2, _K):
            sl = slice(k * _W, (k + 1) * _W)
            nc.gpsimd.tensor_mul(SMSQ[:, sl], SM[:, sl], SM[:, sl])
            q3_last = nc.gpsimd.tensor_add(Q3[:], Q3[:], SMSQ[:, sl])
        # TC = 2.5 - 0.25*Q3 on the otherwise-idle ACT (Identity shares the
        # Sin table set, so no table reload), freeing Pool to reach P2 sooner
        TC = pool.tile([_P, _W], f32)
        nc.scalar.activation(TC[:], Q3[:], ACT.Identity, bias=B25[:], scale=-0.25)
        for k in range(_K):
            sl = slice(k * _W, (k + 1) * _W)
            ws_inst = nc.gpsimd.tensor_scalar_mul(WS[:, sl], SF[:, sl], L[k])
            # order-only edges: keep the whole Q3 tail ahead of WS on Pool
            # so the scheduler doesn't push SMSQ5/Q3 behind WS and delay P2
            tile.add_dep_helper(
                ws_inst.ins, q3_last.ins, sync=False,
                reason="Q3 tail before WS",
            )
        nc.gpsimd.tensor_add(P2[:], WS[:, 0:_W], WS[:, _W : 2 * _W])
        for k in range(2, _K):
            nc.gpsimd.tensor_add(P2[:], P2[:], WS[:, k * _W : (k + 1) * _W])
        nc.vector.reduce_sum(
            Q1[:], WA[:].rearrange("p (k w) -> p w k", k=_K), axis=AX.X
        )

        # denom = 0.5*Q1 - 0.03125*P2^2 / TC with TC = 2.5 - 0.25*Q3.
        # Multiply through by TC to avoid a second reciprocal:
        #   cost = TC / (0.5*Q1*TC - 0.03125*P2^2)   (TC in [1, 2.5] > 0)
        G = pool.tile([_P, _W], f32)
        nc.vector.scalar_tensor_tensor(G[:], Q1[:], 0.5, TC[:], OP.mult, OP.mult)
        TB = pool.tile([_P, _W], f32)
        nc.vector.scalar_tensor_tensor(TB[:], P2[:], 0.03125, P2[:], OP.mult, OP.mult)
        D = pool.tile([_P, _W], f32)
        nc.vector.tensor_sub(D[:], G[:], TB[:])
        WREC = pool.tile([_P, _W], f32)
        nc.vector.reciprocal(WREC[:], D[:])
        COST = pool.tile([_P, _W], f32)
        nc.vector.tensor_mul(COST[:], TC[:], WREC[:])

        colsum = pool.tile([_P, 1], f32)
        nc.vector.reduce_sum(colsum[:], COST[:], axis=AX.X)
        nc.sync.dma_start(out_dram[:], colsum[:])

    nc.compile()
    _CACHE["nc"] = nc
    return nc


def _shard(x):
    # gather the used slice and convert to pi-units in the same pass
    qs = np.asarray(x[:, :_T, 6 : 6 + _K], dtype=np.float32) * np.float32(
        1.0 / np.pi
    )
    return np.ascontiguousarray(
        qs.reshape(_NCORES, _BPC * _T, _K).transpose(0, 2, 1).reshape(
            _NCORES, _K, _P, _W
        )
    )


def _get_runner():
    """Build the jitted 8-core shard_map executable once (mirrors
    bass2jax.run_bass_via_pjrt's multi-core path) so repeat kernel() calls
    skip retracing/recompiling."""
    if "run" in _CACHE:
        return _CACHE["run"]
    import jax
    from jax.sharding import Mesh, PartitionSpec
    from jax.experimental.shard_map import shard_map
    from concourse import bass2jax

    nc = _get_nc()
    bass2jax.install_neuronx_cc_hook()
    assert nc.dbg_addr is None
    pid_name = nc.partition_id_tensor.name if nc.partition_id_tensor else None
    in_names = ("q", "out") + ((pid_name,) if pid_name else ())

    out_aval = jax.core.ShapedArray((_P, 1), np.float32)

    def _body(q, out_zero):
        operands = [q, out_zero]
        if pid_name is not None:
            operands.append(bass2jax.partition_id_tensor())
        (out,) = bass2jax._bass_exec_p.bind(
            *operands,
            out_avals=(out_aval,),
            in_names=in_names,
            out_names=("out",),
            lowering_input_output_aliases=(),
            sim_require_finite=True,
            sim_require_nnan=True,
            nc=nc,
        )
        return (out,)

    devices = jax.devices()[:_NCORES]
    mesh = Mesh(np.asarray(devices), ("core",))
    sharded = jax.jit(
        shard_map(
            _body,
            mesh=mesh,
            in_specs=(PartitionSpec("core"),) * 2,
            out_specs=(PartitionSpec("core"),),
            check_rep=False,
        ),
        donate_argnums=(1,),
        keep_unused=True,
    )

    def run(planes):
        concat_q = planes.reshape(_NCORES * _K, _P, _W)
        zeros = np.zeros((_NCORES * _P, 1), np.float32)
        (out,) = sharded(concat_q, zeros)
        return np.asarray(out)  # (8*128, 1)

    _CACHE["run"] = run
    return run


def _run_library(planes):
    from concourse.bass_utils import run_bass_kernel_spmd

    res = run_bass_kernel_spmd(
        _get_nc(),
        [{"q": planes[i]} for i in range(_NCORES)],
        list(range(_NCORES)),
    )
    return np.stack([r["out"][:, 0] for r in res.results]).astype(np.float32)


def _run_subprocess(planes):
    """Last resort: the accelerator occasionally reports
    NRT_EXEC_UNIT_UNRECOVERABLE; a fresh process reliably recovers it."""
    import os
    import subprocess
    import sys
    import tempfile

    d = tempfile.mkdtemp()
    inp = os.path.join(d, "planes.npy")
    out = os.path.join(d, "out.npy")
    np.save(inp, planes)
    here = os.path.dirname(os.path.abspath(__file__))
    script = (
        "import sys, numpy as np\n"
        f"sys.path.insert(0, {here!r})\n"
        "import kernel as K\n"
        f"planes = np.load({inp!r})\n"
        "out = K._get_runner()(planes)\n"
        f"np.save({out!r}, out)\n"
    )
    err = None
    for _ in range(2):
        try:
            subprocess.run(
                [sys.executable, "-c", script], check=True, timeout=900,
                stdout=subprocess.DEVNULL, stderr=subprocess.DEVNULL,
            )
            return np.load(out).astype(np.float32)
        except Exception as e:  # retry once; device usually recovers
            err = e
    raise err


def kernel(x, cond, time):
    x = np.asarray(x)
    planes = _shard(x)
    try:
        partials = _get_runner()(planes).astype(np.float32)
    except Exception:
        try:
            # library SPMD runner (covers fast-path/jax API drift)
            partials = _run_library(planes)
        except Exception:
            # fresh process recovers a wedged accelerator
            partials = _run_subprocess(planes)
    return np.float32(partials.sum(dtype=np.float32))



# revision 6
# speedup vs baseline: 1.2149x; 1.2149x over previous
"""Trainium2 Bass kernel for nn_CostFn_18562848653837.

reference(x, cond, time) only reads x[b, j, 6+k] for j in [0,26), k in [0,6)
(~2.6 MB of the 436 MB input; cond/time are unused) and computes, per point,
the reflected mass 1 / (u^T J M^{-1} J^T u) with u = e_x, which reduces via
Sherman-Morrison (M = 2I + 0.5 c c^T, c = cos(cq), s = sin(cq), v = L*s,
cq = cumsum(q)) to

    denom = 0.25*(SL2 - A) - 0.03125*P2^2 / t,   t = 1.75 + 0.125*B

with the double-angle identities (SL2 = sum L_k^2):

    A  = sum_k L_k^2 cos(2 cq_k)
    B  = sum_k cos(2 cq_k)
    P2 = sum_k L_k sin(2 cq_k)

so only sin/cos of the SINGLE angle 2*cq_k is needed per plane, and
cost = t / (0.25*(SL2 - A)*t - 0.03125*P2^2) needs one divide, no extra
reciprocal.

The host ships AFFINE-prepped planes (same class as the baseline's 1/pi
scale): g_k = cumsum(q)_k / pi + 4.25, as float16 (quantization error is
random-sign across 13312 points/core; the summed rel-err contribution is
~1e-4, far under the 2e-2 gate). On device, one fused tensor_scalar per
plane produces each trig input:

    m'_k = g_k mod 1          -> Sin(2pi*m' - pi) = -cos(2 cq)   (the +0.25
                                 host offset turns sin into cos)
    m_k  = (g_k - 0.25) mod 1 -> Sin(2pi*m  - pi) = -sin(2 cq)

(mod semantics: np.remainder, result in [0,1); the +4 host bias keeps g > 0
so floored and truncated mod agree). Both ACT calls share one scale/bias
(2pi shaded by one ulp, bias -pi), keeping the Sin table domain satisfied.

A/P2 are Horner chains of scalar_tensor_tensor ops (weight ratios folded
into the chain, tail weights folded into downstream fused scalars); B is a
pairwise tree. Work is spread: mods + A + P2 + final chain on Pool, mods +
B-tree + t/U/G1 on DVE, the two Sins on ACT (with a dep-free warm-up Sin so
the table load hides behind the input DMAs).

Sharding: pure data parallel over batch - core i gets batches
[512*i, 512*(i+1)), i.e. 512*26 = 13312 points laid out as a (128, 104) tile
per plane, planes contiguous along free (128, 624), fp16, shipped as two
128-descriptor DMAs (624B/partition each, no sub-512B latency penalty).
Each core emits one f32 (128,1) partial-sum column; host adds the 8*128.
"""

import numpy as np

_P, _W, _K = 128, 104, 6
_F = _K * _W
_NCORES = 8
_B, _H, _T = 4096, 1024, 26
_BPC = _B // _NCORES  # batches per core

_CACHE = {}

_LV = np.arange(1, 7, dtype=np.float64) * 0.1 + 0.3  # [0.4 .. 0.9]
_SL2 = float((_LV * _LV).sum())  # 2.71


def _get_nc():
    if "nc" in _CACHE:
        return _CACHE["nc"]

    import concourse.tile as tile
    import concourse.mybir as mybir
    from concourse import bacc

    PI32 = float(np.float32(np.pi))
    # One-ulp-shaded 2*pi: m in [0,1) so SCALE2*m - pi stays strictly inside
    # the Sin table domain [-pi, pi].
    SCALE2 = float(np.float32(2.0 * np.pi * (1.0 - 2.0**-23)))
    L = [float(v) for v in _LV]
    W2 = [float(v * v) for v in _LV]

    f32 = mybir.dt.float32
    f16 = mybir.dt.float16
    AX = mybir.AxisListType
    OP = mybir.AluOpType
    ACT = mybir.ActivationFunctionType

    nc = bacc.Bacc(
        "TRN2", target_bir_lowering=False, debug=False, num_devices=_NCORES,
        disable_frame_to_traceback=True,
    )
    q_dram = nc.dram_tensor("q", [_P, _F], f16, kind="ExternalInput")
    out_dram = nc.dram_tensor("out", [_P, 1], f32, kind="ExternalOutput")

    with (
        tile.TileContext(nc) as tc,
        tc.tile_pool(name="pool", bufs=1) as pool,
    ):
        # constant -pi bias tile for both ACT Sins, built while DMAs fly
        BNEG = pool.tile([_P, 1], f32)
        nc.vector.memset(BNEG[:], -PI32)

        # Dep-free dummy Sin on the pre-initialized const-1.0 AP: hoists the
        # ~1.3us Sin table-set load to t~0, hidden behind the input DMAs.
        one_ap = nc.const_aps.aps[(f32, 1.0)]
        WARM = pool.tile([_P, 1], f32)
        nc.scalar.activation(WARM[:], one_ap[:_P], ACT.Sin)

        # input: 6 fp16 planes, 2 DMAs x 3 planes (624B/partition/descriptor)
        Q = pool.tile([_P, _F], f16)
        nc.gpsimd.dma_start(Q[:, 0 : 3 * _W], q_dram[:, 0 : 3 * _W])
        nc.sync.dma_start(Q[:, 3 * _W : _F], q_dram[:, 3 * _W : _F])

        # range reduction: m' = g mod 1 (cos path), m = (g - 0.25) mod 1
        # (sin path); one fused tensor_scalar per plane, fp16 in / f32 out.
        # All six m' land first (they gate the first ACT call).
        MP = pool.tile([_P, _F], f32)
        MS = pool.tile([_P, _F], f32)

        def sl(k):
            return slice(k * _W, (k + 1) * _W)

        # Pool: planes 0,1 from DMA1 then 3,4 from DMA2; DVE: 2 then 5.
        nc.gpsimd.tensor_scalar(MP[:, sl(0)], Q[:, sl(0)], 1.0, None, OP.mod)
        nc.gpsimd.tensor_scalar(MP[:, sl(1)], Q[:, sl(1)], 1.0, None, OP.mod)
        nc.vector.tensor_scalar(MP[:, sl(2)], Q[:, sl(2)], 1.0, None, OP.mod)
        nc.gpsimd.tensor_scalar(MP[:, sl(3)], Q[:, sl(3)], 1.0, None, OP.mod)
        nc.gpsimd.tensor_scalar(MP[:, sl(4)], Q[:, sl(4)], 1.0, None, OP.mod)
        nc.vector.tensor_scalar(MP[:, sl(5)], Q[:, sl(5)], 1.0, None, OP.mod)
        for k, eng in zip(range(_K), ["g", "g", "v", "g", "g", "v"]):
            e = nc.gpsimd if eng == "g" else nc.vector
            e.tensor_scalar(MS[:, sl(k)], Q[:, sl(k)], -0.25, 1.0, OP.add, OP.mod)

        # CF = -cos(2 cq), SF = -sin(2 cq)
        CF = pool.tile([_P, _F], f32)
        nc.scalar.activation(CF[:], MP[:], ACT.Sin, bias=BNEG[:], scale=SCALE2)
        SF = pool.tile([_P, _F], f32)
        nc.scalar.activation(SF[:], MS[:], ACT.Sin, bias=BNEG[:], scale=SCALE2)

        # A-Horner on Pool: h_k = h_{k-1} * (w_{k-1}/w_k) + CF_k, w = L^2
        HA = [pool.tile([_P, _W], f32, name=f"ha{i}") for i in range(2)]
        nc.gpsimd.scalar_tensor_tensor(
            HA[1][:], CF[:, sl(0)], W2[0] / W2[1], CF[:, sl(1)], OP.mult, OP.add
        )
        for k in range(2, _K):
            nc.gpsimd.scalar_tensor_tensor(
                HA[k % 2][:], HA[(k - 1) % 2][:], W2[k - 1] / W2[k],
                CF[:, sl(k)], OP.mult, OP.add,
            )
        # U = 0.25*(SL2 + A_dev) = h5 * 0.25*w5 + 0.25*SL2   (on DVE)
        U = pool.tile([_P, _W], f32)
        nc.vector.tensor_scalar(
            U[:], HA[(_K - 1) % 2][:], 0.25 * W2[_K - 1], 0.25 * _SL2,
            OP.mult, OP.add,
        )

        # B pairwise tree on DVE: b1 = CF[0:3] + CF[3:6] (312 wide)
        B1 = pool.tile([_P, 3 * _W], f32)
        nc.vector.tensor_add(B1[:], CF[:, 0 : 3 * _W], CF[:, 3 * _W : _F])
        B2 = pool.tile([_P, _W], f32)
        nc.vector.tensor_add(B2[:], B1[:, 0:_W], B1[:, _W : 2 * _W])
        B3 = pool.tile([_P, _W], f32)
        nc.vector.tensor_add(B3[:], B2[:], B1[:, 2 * _W : 3 * _W])
        # t = 1.75 - 0.125*B_dev  (DVE)
        TT = pool.tile([_P, _W], f32)
        nc.vector.tensor_scalar(TT[:], B3[:], -0.125, 1.75, OP.mult, OP.add)
        # G1 = U * t  (DVE)
        G1 = pool.tile([_P, _W], f32)
        nc.vector.tensor_mul(G1[:], U[:], TT[:])

        # P2-Horner on Pool over SF planes (weights L_k)
        HP = [pool.tile([_P, _W], f32, name=f"hp{i}") for i in range(2)]
        nc.gpsimd.scalar_tensor_tensor(
            HP[1][:], SF[:, sl(0)], L[0] / L[1], SF[:, sl(1)], OP.mult, OP.add
        )
        for k in range(2, _K):
            nc.gpsimd.scalar_tensor_tensor(
                HP[k % 2][:], HP[(k - 1) % 2][:], L[k - 1] / L[k],
                SF[:, sl(k)], OP.mult, OP.add,
            )
        # G2 = 0.03125 * P2_true^2 = (0.03125*L5^2 * p5) * p5  (Pool)
        G2 = pool.tile([_P, _W], f32)
        p5 = HP[(_K - 1) % 2]
        nc.gpsimd.scalar_tensor_tensor(
            G2[:], p5[:], 0.03125 * L[_K - 1] * L[_K - 1], p5[:],
            OP.mult, OP.mult,
        )
        # G = G1 - G2 (Pool); fused cost = t / G and row-sum on DVE
        G = pool.tile([_P, _W], f32)
        nc.gpsimd.tensor_sub(G[:], G1[:], G2[:])
        C = pool.tile([_P, _W], f32)
        colsum = pool.tile([_P, 1], f32)
        nc.vector.tensor_tensor_reduce(
            C[:], TT[:], G[:], 1.0, 0.0, OP.divide, OP.add, colsum[:]
        )

        nc.sync.dma_start(out_dram[:], colsum[:])

    nc.compile()
    _CACHE["nc"] = nc
    return nc


def _shard(x):
    # gather the used slice; prefix-sum over joints, scale to pi-units and
    # bias (+4 keeps g positive for mod; +0.25 turns the first Sin into cos)
    qs = np.asarray(x[:, :_T, 6 : 6 + _K], dtype=np.float32)
    g = np.cumsum(qs, axis=-1, dtype=np.float32) * np.float32(1.0 / np.pi)
    g += np.float32(4.25)
    # (ncores, 128, 104, 6) -> planes contiguous along free: (.., 6, 128, 104)
    g = g.reshape(_NCORES, _BPC * _T, _K).transpose(0, 2, 1).reshape(
        _NCORES, _K, _P, _W
    )
    # interleave planes along free axis per partition: (ncores, 128, 624)
    g = np.ascontiguousarray(g.transpose(0, 2, 1, 3)).reshape(
        _NCORES, _P, _F
    )
    return g.astype(np.float16)


def _get_runner():
    """Build the jitted 8-core shard_map executable once (mirrors
    bass2jax.run_bass_via_pjrt's multi-core path) so repeat kernel() calls
    skip retracing/recompiling."""
    if "run" in _CACHE:
        return _CACHE["run"]
    import jax
    from jax.sharding import Mesh, PartitionSpec
    from jax.experimental.shard_map import shard_map
    from concourse import bass2jax

    nc = _get_nc()
    bass2jax.install_neuronx_cc_hook()
    assert nc.dbg_addr is None
    pid_name = nc.partition_id_tensor.name if nc.partition_id_tensor else None
    in_names = ("q", "out") + ((pid_name,) if pid_name else ())

    out_aval = jax.core.ShapedArray((_P, 1), np.float32)

    def _body(q, out_zero):
        operands = [q, out_zero]
        if pid_name is not None:
            operands.append(bass2jax.partition_id_tensor())
        (out,) = bass2jax._bass_exec_p.bind(
            *operands,
            out_avals=(out_aval,),
            in_names=in_names,
            out_names=("out",),
            lowering_input_output_aliases=(),
            sim_require_finite=True,
            sim_require_nnan=True,
            nc=nc,
        )
        return (out,)

    devices = jax.devices()[:_NCORES]
    mesh = Mesh(np.asarray(devices), ("core",))
    sharded = jax.jit(
        shard_map(
            _body,
            mesh=mesh,
            in_specs=(PartitionSpec("core"),) * 2,
            out_specs=(PartitionSpec("core"),),
            check_rep=False,
        ),
        donate_argnums=(1,),
        keep_unused=True,
    )

    def run(planes):
        concat_q = planes.reshape(_NCORES * _P, _F)
        zeros = np.zeros((_NCORES * _P, 1), np.float32)
        (out,) = sharded(concat_q, zeros)
        return np.asarray(out)  # (8*128, 1)

    _CACHE["run"] = run
    return run


def _run_library(planes):
    from concourse.bass_utils import run_bass_kernel_spmd

    res = run_bass_kernel_spmd(
        _get_nc(),
        [{"q": planes[i]} for i in range(_NCORES)],
        list(range(_NCORES)),
    )
    return np.stack([r["out"][:, 0] for r in res.results]).astype(np.float32)


def _run_subprocess(planes):
    """Last resort: the accelerator occasionally reports
    NRT_EXEC_UNIT_UNRECOVERABLE; a fresh process reliably recovers it."""
    import os
    import subprocess
    import sys
    import tempfile

    d = tempfile.mkdtemp()
    inp = os.path.join(d, "planes.npy")
    out = os.path.join(d, "out.npy")
    np.save(inp, planes)
    here = os.path.dirname(os.path.abspath(__file__))
    script = (
        "import sys, numpy as np\n"
        f"sys.path.insert(0, {here!r})\n"
        "import kernel as K\n"
        f"planes = np.load({inp!r})\n"
        "out = K._get_runner()(planes)\n"
        f"np.save({out!r}, out)\n"
    )
    err = None
    for _ in range(2):
        try:
            subprocess.run(
                [sys.executable, "-c", script], check=True, timeout=900,
                stdout=subprocess.DEVNULL, stderr=subprocess.DEVNULL,
            )
            return np.load(out).astype(np.float32)
        except Exception as e:  # retry once; device usually recovers
            err = e
    raise err


def kernel(x, cond, time):
    x = np.asarray(x)
    planes = _shard(x)
    try:
        partials = _get_runner()(planes).astype(np.float32)
    except Exception:
        try:
            # library SPMD runner (covers fast-path/jax API drift)
            partials = _run_library(planes)
        except Exception:
            # fresh process recovers a wedged accelerator
            partials = _run_subprocess(planes)
    return np.float32(partials.sum(dtype=np.float32))


# revision 14
# speedup vs baseline: 1.6133x; 1.3280x over previous
"""Trainium2 Bass kernel for nn_CostFn_18562848653837.

reference(x, cond, time) only reads x[b, j, 6+k] for j in [0,26), k in [0,6)
(~2.6 MB of the 436 MB input; cond/time are unused) and computes, per point,
the reflected mass 1 / (u^T J M^{-1} J^T u) with u = e_x, which reduces via
Sherman-Morrison (M = 2I + 0.5 c c^T, c = cos(cq), s = sin(cq), v = L*s,
cq = cumsum(q)) to

    denom = 0.25*(SL2 - A) - 0.03125*P2^2 / t,   t = 1.75 + 0.125*B

with the double-angle identities (SL2 = sum L_k^2):

    A  = sum_k L_k^2 cos(2 cq_k)
    B  = sum_k cos(2 cq_k)
    P2 = sum_k L_k sin(2 cq_k)

so only sin/cos of the SINGLE angle 2*cq_k is needed per plane, and
cost = t / (0.25*(SL2 - A)*t - 0.03125*P2^2) needs one divide, no extra
reciprocal.

The host ships AFFINE-prepped planes (same class as the baseline's 1/pi
scale): g_k = cumsum(q)_k / pi + 4.25, as float16 (quantization error is
random-sign across 13312 points/core; the summed rel-err contribution is
~1e-4, far under the 2e-2 gate). On device, one fused tensor_scalar per
plane produces each trig input:

    m'_k = g_k mod 1          -> Sin(2pi*m' - pi) = -cos(2 cq)   (the +0.25
                                 host offset turns sin into cos)
    m_k  = (g_k - 0.25) mod 1 -> Sin(2pi*m  - pi) = -sin(2 cq)

(mod semantics: np.remainder, result in [0,1); the +4 host bias keeps g > 0
so floored and truncated mod agree). Both ACT calls share one scale/bias
(2pi shaded by one ulp, bias -pi), keeping the Sin table domain satisfied.

A/P2 are Horner chains of scalar_tensor_tensor ops (weight ratios folded
into the chain, tail weights folded into downstream fused scalars); B is a
pairwise tree. Work is spread: mods + A + P2 + final chain on Pool, mods +
B-tree + t/U/G1 on DVE, the two Sins on ACT (with a dep-free warm-up Sin so
the table load hides behind the input DMAs).

Sharding: pure data parallel over batch - core i gets batches
[512*i, 512*(i+1)), i.e. 512*26 = 13312 points laid out as a (128, 104) tile
per plane, planes contiguous along free (128, 624), fp16, shipped as two
128-descriptor DMAs (624B/partition each, no sub-512B latency penalty).
Each core emits one f32 (128,1) partial-sum column; host adds the 8*128.
"""

import numpy as np

_P, _W, _K = 128, 104, 6
_F = _K * _W
_NCORES = 8
_B, _H, _T = 4096, 1024, 26
_BPC = _B // _NCORES  # batches per core

_CACHE = {}
_SCATTER_OUT = True  # output via SWDGE prepare/trigger instead of dma_start

_LV = np.arange(1, 7, dtype=np.float64) * 0.1 + 0.3  # [0.4 .. 0.9]
_SL2 = float((_LV * _LV).sum())  # 2.71


def _oidx():
    # identity scatter indices, wrapped in 16 partitions: idx i at
    # [i % 16, i // 16]; executor reads a 128-partition view, rows 16+ unused
    idx = np.zeros((_P, 8), dtype=np.int16)
    idx[:16] = np.arange(_P, dtype=np.int16).reshape(8, 16).T
    return idx


def _get_nc():
    if "nc" in _CACHE:
        return _CACHE["nc"]

    import concourse.tile as tile
    import concourse.mybir as mybir
    from concourse import bacc

    PI32 = float(np.float32(np.pi))
    # One-ulp-shaded 2*pi: m in [0,1) so SCALE2*m - pi stays strictly inside
    # the Sin table domain [-pi, pi].
    SCALE2 = float(np.float32(2.0 * np.pi * (1.0 - 2.0**-23)))
    L = [float(v) for v in _LV]
    W2 = [float(v * v) for v in _LV]

    f32 = mybir.dt.float32
    f16 = mybir.dt.float16
    AX = mybir.AxisListType
    OP = mybir.AluOpType
    ACT = mybir.ActivationFunctionType

    nc = bacc.Bacc(
        "TRN2", target_bir_lowering=False, debug=False, num_devices=_NCORES,
        disable_frame_to_traceback=True,
    )
    q_dram = nc.dram_tensor("q", [_P, _F], f16, kind="ExternalInput")
    if _SCATTER_OUT:
        # scatter-add needs a 256B-multiple dram row stride -> pad to 64 f32
        out_dram = nc.dram_tensor("out", [_P, 64], f32, kind="ExternalOutput")
        idx_dram = nc.dram_tensor("oidx", [128, 8], mybir.dt.int16,
                                  kind="ExternalInput")
    else:
        out_dram = nc.dram_tensor("out", [_P, 1], f32, kind="ExternalOutput")

    with (
        tile.TileContext(nc) as tc,
        tc.tile_pool(name="pool", bufs=1) as pool,
    ):
        # constant -pi bias tile for both ACT Sins, built while DMAs fly
        BNEG = pool.tile([_P, 1], f32)
        nc.vector.memset(BNEG[:], -PI32)

        # Dep-free dummy Sin on the pre-initialized const-1.0 AP: hoists the
        # ~1.3us Sin table-set load to t~0, hidden behind the input DMAs.
        one_ap = nc.const_aps.aps[(f32, 1.0)]
        WARM = pool.tile([_P, 1], f32)
        nc.scalar.activation(WARM[:], one_ap[:_P], ACT.Sin)

        # input: 6 fp16 planes, 2 DMAs x 3 planes (624B/partition/descriptor)
        Q = pool.tile([_P, _F], f16)
        nc.gpsimd.dma_start(Q[:, 0 : 3 * _W], q_dram[:, 0 : 3 * _W])
        nc.sync.dma_start(Q[:, 3 * _W : _F], q_dram[:, 3 * _W : _F])
        if _SCATTER_OUT:
            OIDX = pool.tile([128, 8], mybir.dt.int16)
            nc.sync.dma_start(OIDX[:], idx_dram[:])

        # range reduction: m' = g mod 1 (cos path), m = (g - 0.25) mod 1
        # (sin path); one fused tensor_scalar per plane, fp16 in / f32 out.
        # All six m' land first (they gate the first ACT call).
        MP = pool.tile([_P, _F], f32)
        MS = pool.tile([_P, _F], f32)

        def sl(k):
            return slice(k * _W, (k + 1) * _W)

        # Pool: planes 0,1 from DMA1 then 3,4 from DMA2; DVE: 2 then 5.
        nc.gpsimd.tensor_scalar(MP[:, sl(0)], Q[:, sl(0)], 1.0, None, OP.mod)
        nc.gpsimd.tensor_scalar(MP[:, sl(1)], Q[:, sl(1)], 1.0, None, OP.mod)
        nc.vector.tensor_scalar(MP[:, sl(2)], Q[:, sl(2)], 1.0, None, OP.mod)
        nc.gpsimd.tensor_scalar(MP[:, sl(3)], Q[:, sl(3)], 1.0, None, OP.mod)
        nc.gpsimd.tensor_scalar(MP[:, sl(4)], Q[:, sl(4)], 1.0, None, OP.mod)
        nc.vector.tensor_scalar(MP[:, sl(5)], Q[:, sl(5)], 1.0, None, OP.mod)
        for k, eng in zip(range(_K), ["g", "g", "v", "g", "g", "v"]):
            e = nc.gpsimd if eng == "g" else nc.vector
            e.tensor_scalar(MS[:, sl(k)], Q[:, sl(k)], -0.25, 1.0, OP.add, OP.mod)

        # CF = -cos(2 cq), SF = -sin(2 cq)
        CF = pool.tile([_P, _F], f32)
        nc.scalar.activation(CF[:], MP[:], ACT.Sin, bias=BNEG[:], scale=SCALE2)
        SF = pool.tile([_P, _F], f32)
        nc.scalar.activation(SF[:], MS[:], ACT.Sin, bias=BNEG[:], scale=SCALE2)

        # A-Horner on Pool: h_k = h_{k-1} * (w_{k-1}/w_k) + CF_k, w = L^2
        HA = [pool.tile([_P, _W], f32, name=f"ha{i}") for i in range(2)]
        nc.gpsimd.scalar_tensor_tensor(
            HA[1][:], CF[:, sl(0)], W2[0] / W2[1], CF[:, sl(1)], OP.mult, OP.add
        )
        for k in range(2, _K):
            nc.gpsimd.scalar_tensor_tensor(
                HA[k % 2][:], HA[(k - 1) % 2][:], W2[k - 1] / W2[k],
                CF[:, sl(k)], OP.mult, OP.add,
            )
        # U = 0.25*(SL2 + A_dev) = h5 * 0.25*w5 + 0.25*SL2   (on DVE)
        U = pool.tile([_P, _W], f32)
        nc.vector.tensor_scalar(
            U[:], HA[(_K - 1) % 2][:], 0.25 * W2[_K - 1], 0.25 * _SL2,
            OP.mult, OP.add,
        )

        # B pairwise tree on DVE: b1 = CF[0:3] + CF[3:6] (312 wide)
        B1 = pool.tile([_P, 3 * _W], f32)
        nc.vector.tensor_add(B1[:], CF[:, 0 : 3 * _W], CF[:, 3 * _W : _F])
        B2 = pool.tile([_P, _W], f32)
        nc.vector.tensor_add(B2[:], B1[:, 0:_W], B1[:, _W : 2 * _W])
        B3 = pool.tile([_P, _W], f32)
        nc.vector.tensor_add(B3[:], B2[:], B1[:, 2 * _W : 3 * _W])
        # t = 1.75 - 0.125*B_dev  (DVE)
        TT = pool.tile([_P, _W], f32)
        nc.vector.tensor_scalar(TT[:], B3[:], -0.125, 1.75, OP.mult, OP.add)
        # G1 = U * t  (DVE)
        G1 = pool.tile([_P, _W], f32)
        nc.vector.tensor_mul(G1[:], U[:], TT[:])

        # P2-Horner on Pool over SF planes (weights L_k)
        HP = [pool.tile([_P, _W], f32, name=f"hp{i}") for i in range(2)]
        nc.gpsimd.scalar_tensor_tensor(
            HP[1][:], SF[:, sl(0)], L[0] / L[1], SF[:, sl(1)], OP.mult, OP.add
        )
        for k in range(2, _K):
            nc.gpsimd.scalar_tensor_tensor(
                HP[k % 2][:], HP[(k - 1) % 2][:], L[k - 1] / L[k],
                SF[:, sl(k)], OP.mult, OP.add,
            )
        # G2 = 0.03125 * P2_true^2 = (0.03125*L5^2 * p5) * p5  (Pool)
        G2 = pool.tile([_P, _W], f32)
        p5 = HP[(_K - 1) % 2]
        nc.gpsimd.scalar_tensor_tensor(
            G2[:], p5[:], 0.03125 * L[_K - 1] * L[_K - 1], p5[:],
            OP.mult, OP.mult,
        )
        # G = G1 - G2 (Pool); fused cost = t / G and row-sum on DVE
        G = pool.tile([_P, _W], f32)
        nc.gpsimd.tensor_sub(G[:], G1[:], G2[:])
        C = pool.tile([_P, _W], f32)
        colsum = pool.tile([_P, 1], f32)
        nc.vector.tensor_tensor_reduce(
            C[:], TT[:], G[:], 1.0, 0.0, OP.divide, OP.add, colsum[:]
        )

        if _SCATTER_OUT:
            # prep runs early (descriptor gen only); the colsum data dep
            # moves to the cheap trigger, skipping the 500ns DGE config and
            # 650ns DGE->DMA delay on the critical tail
            dma_sem = nc.alloc_semaphore("swdge_dma")
            nc.gpsimd.dma_scatter_add(
                out_dram[:, 0:1], colsum[:], OIDX[:], num_idxs=_P,
                num_idxs_reg=_P, elem_size=1, elem_step=64, prepare_only=True,
                sem=dma_sem,
            )
            nc.gpsimd.trigger_dma(count=None)
        else:
            nc.sync.dma_start(out_dram[:], colsum[:])

    nc.compile()
    _CACHE["nc"] = nc
    return nc


def _shard(x):
    # gather the used slice; prefix-sum over joints, scale to pi-units and
    # bias (+4 keeps g positive for mod; +0.25 turns the first Sin into cos)
    qs = np.asarray(x[:, :_T, 6 : 6 + _K], dtype=np.float32)
    g = np.cumsum(qs, axis=-1, dtype=np.float32) * np.float32(1.0 / np.pi)
    g += np.float32(4.25)
    # (ncores, 128, 104, 6) -> planes contiguous along free: (.., 6, 128, 104)
    g = g.reshape(_NCORES, _BPC * _T, _K).transpose(0, 2, 1).reshape(
        _NCORES, _K, _P, _W
    )
    # interleave planes along free axis per partition: (ncores, 128, 624)
    g = np.ascontiguousarray(g.transpose(0, 2, 1, 3)).reshape(
        _NCORES, _P, _F
    )
    return g.astype(np.float16)


def _get_runner():
    """Build the jitted 8-core shard_map executable once (mirrors
    bass2jax.run_bass_via_pjrt's multi-core path) so repeat kernel() calls
    skip retracing/recompiling."""
    if "run" in _CACHE:
        return _CACHE["run"]
    import jax
    from jax.sharding import Mesh, PartitionSpec
    from jax.experimental.shard_map import shard_map
    from concourse import bass2jax

    nc = _get_nc()
    bass2jax.install_neuronx_cc_hook()
    assert nc.dbg_addr is None
    pid_name = nc.partition_id_tensor.name if nc.partition_id_tensor else None
    in_names = ("q", "out") + ((pid_name,) if pid_name else ())

    out_aval = jax.core.ShapedArray((_P, 1), np.float32)

    def _body(q, out_zero):
        operands = [q, out_zero]
        if pid_name is not None:
            operands.append(bass2jax.partition_id_tensor())
        (out,) = bass2jax._bass_exec_p.bind(
            *operands,
            out_avals=(out_aval,),
            in_names=in_names,
            out_names=("out",),
            lowering_input_output_aliases=(),
            sim_require_finite=True,
            sim_require_nnan=True,
            nc=nc,
        )
        return (out,)

    devices = jax.devices()[:_NCORES]
    mesh = Mesh(np.asarray(devices), ("core",))
    sharded = jax.jit(
        shard_map(
            _body,
            mesh=mesh,
            in_specs=(PartitionSpec("core"),) * 2,
            out_specs=(PartitionSpec("core"),),
            check_rep=False,
        ),
        donate_argnums=(1,),
        keep_unused=True,
    )

    def run(planes):
        concat_q = planes.reshape(_NCORES * _P, _F)
        zeros = np.zeros((_NCORES * _P, 1), np.float32)
        (out,) = sharded(concat_q, zeros)
        return np.asarray(out)  # (8*128, 1)

    _CACHE["run"] = run
    return run


def _run_library(planes):
    from concourse.bass_utils import run_bass_kernel_spmd

    res = run_bass_kernel_spmd(
        _get_nc(),
        [{"q": planes[i]} for i in range(_NCORES)],
        list(range(_NCORES)),
    )
    return np.stack([r["out"][:, 0] for r in res.results]).astype(np.float32)


def _run_subprocess(planes):
    """Last resort: the accelerator occasionally reports
    NRT_EXEC_UNIT_UNRECOVERABLE; a fresh process reliably recovers it."""
    import os
    import subprocess
    import sys
    import tempfile

    d = tempfile.mkdtemp()
    inp = os.path.join(d, "planes.npy")
    out = os.path.join(d, "out.npy")
    np.save(inp, planes)
    here = os.path.dirname(os.path.abspath(__file__))
    script = (
        "import sys, numpy as np\n"
        f"sys.path.insert(0, {here!r})\n"
        "import kernel as K\n"
        f"planes = np.load({inp!r})\n"
        "out = K._get_runner()(planes)\n"
        f"np.save({out!r}, out)\n"
    )
    err = None
    for _ in range(2):
        try:
            subprocess.run(
                [sys.executable, "-c", script], check=True, timeout=900,
                stdout=subprocess.DEVNULL, stderr=subprocess.DEVNULL,
            )
            return np.load(out).astype(np.float32)
        except Exception as e:  # retry once; device usually recovers
            err = e
    raise err


def kernel(x, cond, time):
    x = np.asarray(x)
    planes = _shard(x)
    try:
        partials = _get_runner()(planes).astype(np.float32)
    except Exception:
        try:
            # library SPMD runner (covers fast-path/jax API drift)
            partials = _run_library(planes)
        except Exception:
            # fresh process recovers a wedged accelerator
            partials = _run_subprocess(planes)
    return np.float32(partials.sum(dtype=np.float32))


# revision 17
# speedup vs baseline: 1.6435x; 1.0187x over previous
"""Trainium2 Bass kernel for nn_CostFn_18562848653837.

reference(x, cond, time) only reads x[b, j, 6+k] for j in [0,26), k in [0,6)
(~2.6 MB of the 436 MB input; cond/time are unused) and computes, per point,
the reflected mass 1 / (u^T J M^{-1} J^T u) with u = e_x, which reduces via
Sherman-Morrison (M = 2I + 0.5 c c^T, c = cos(cq), s = sin(cq), v = L*s,
cq = cumsum(q)) to

    denom = 0.25*(SL2 - A) - 0.03125*P2^2 / t,   t = 1.75 + 0.125*B

with the double-angle identities (SL2 = sum L_k^2):

    A  = sum_k L_k^2 cos(2 cq_k)
    B  = sum_k cos(2 cq_k)
    P2 = sum_k L_k sin(2 cq_k)

so only sin/cos of the SINGLE angle 2*cq_k is needed per plane, and
cost = t / (0.25*(SL2 - A)*t - 0.03125*P2^2) needs one divide, no extra
reciprocal.

The host ships AFFINE-prepped planes (same class as the baseline's 1/pi
scale): g_k = cumsum(q)_k / pi + 4.25, as float16 (quantization error is
random-sign across 13312 points/core; the summed rel-err contribution is
~1e-4, far under the 2e-2 gate). On device, one fused tensor_scalar per
plane produces each trig input:

    m'_k = g_k mod 1          -> Sin(2pi*m' - pi) = -cos(2 cq)   (the +0.25
                                 host offset turns sin into cos)
    m_k  = (g_k - 0.25) mod 1 -> Sin(2pi*m  - pi) = -sin(2 cq)

(mod semantics: np.remainder, result in [0,1); the +4 host bias keeps g > 0
so floored and truncated mod agree). Both ACT calls share one scale/bias
(2pi shaded by one ulp, bias -pi), keeping the Sin table domain satisfied.

A/P2 are Horner chains of scalar_tensor_tensor ops (weight ratios folded
into the chain, tail weights folded into downstream fused scalars); B is a
pairwise tree. Work is spread: mods + A + P2 + final chain on Pool, mods +
B-tree + t/U/G1 on DVE, the two Sins on ACT (with a dep-free warm-up Sin so
the table load hides behind the input DMAs).

Sharding: pure data parallel over batch - core i gets batches
[512*i, 512*(i+1)), i.e. 512*26 = 13312 points laid out as a (128, 104) tile
per plane, planes contiguous along free (128, 624), fp16, shipped as two
128-descriptor DMAs (624B/partition each, no sub-512B latency penalty).
Each core emits one f32 (128,1) partial-sum column; host adds the 8*128.
"""

import numpy as np

_P, _W, _K = 128, 104, 6
_F = _K * _W
_NCORES = 8
_B, _H, _T = 4096, 1024, 26
_BPC = _B // _NCORES  # batches per core

_CACHE = {}
_SCATTER_OUT = True  # output via SWDGE prepare/trigger instead of dma_start

_LV = np.arange(1, 7, dtype=np.float64) * 0.1 + 0.3  # [0.4 .. 0.9]
_SL2 = float((_LV * _LV).sum())  # 2.71


def _oidx():
    # identity scatter indices, wrapped in 16 partitions: idx i at
    # [i % 16, i // 16]; executor reads a 128-partition view, rows 16+ unused
    idx = np.zeros((_P, 8), dtype=np.int16)
    idx[:16] = np.arange(_P, dtype=np.int16).reshape(8, 16).T
    return idx


def _get_nc():
    if "nc" in _CACHE:
        return _CACHE["nc"]

    import concourse.tile as tile
    import concourse.mybir as mybir
    from concourse import bacc

    PI32 = float(np.float32(np.pi))
    # One-ulp-shaded 2*pi: m in [0,1) so SCALE2*m - pi stays strictly inside
    # the Sin table domain [-pi, pi].
    SCALE2 = float(np.float32(2.0 * np.pi * (1.0 - 2.0**-23)))
    L = [float(v) for v in _LV]
    W2 = [float(v * v) for v in _LV]

    f32 = mybir.dt.float32
    f16 = mybir.dt.float16
    AX = mybir.AxisListType
    OP = mybir.AluOpType
    ACT = mybir.ActivationFunctionType

    nc = bacc.Bacc(
        "TRN2", target_bir_lowering=False, debug=False, num_devices=_NCORES,
        disable_frame_to_traceback=True,
    )
    q_dram = nc.dram_tensor("q", [_P, _F], f16, kind="ExternalInput")
    if _SCATTER_OUT:
        # scatter-add needs a 256B-multiple dram row stride -> pad to 64 f32
        out_dram = nc.dram_tensor("out", [_P, 64], f32, kind="ExternalOutput")
        idx_dram = nc.dram_tensor("oidx", [128, 8], mybir.dt.int16,
                                  kind="ExternalInput")
    else:
        out_dram = nc.dram_tensor("out", [_P, 1], f32, kind="ExternalOutput")

    with (
        tile.TileContext(nc) as tc,
        tc.tile_pool(name="pool", bufs=1) as pool,
    ):
        # constant -pi bias tile for both ACT Sins, built while DMAs fly
        BNEG = pool.tile([_P, 1], f32)
        nc.vector.memset(BNEG[:], -PI32)

        # Dep-free dummy Sin on the pre-initialized const-1.0 AP: hoists the
        # ~1.3us Sin table-set load to t~0, hidden behind the input DMAs.
        one_ap = nc.const_aps.aps[(f32, 1.0)]
        WARM = pool.tile([_P, 1], f32)
        nc.scalar.activation(WARM[:], one_ap[:_P], ACT.Sin)

        # input: 6 fp16 planes, 2 DMAs x 3 planes (624B/partition/descriptor)
        Q = pool.tile([_P, _F], f16)
        nc.gpsimd.dma_start(Q[:, 0 : 3 * _W], q_dram[:, 0 : 3 * _W])
        nc.sync.dma_start(Q[:, 3 * _W : _F], q_dram[:, 3 * _W : _F])
        if _SCATTER_OUT:
            OIDX = pool.tile([128, 8], mybir.dt.int16)
            nc.sync.dma_start(OIDX[:], idx_dram[:])

        # range reduction: m' = g mod 1 (cos path), m = (g - 0.25) mod 1
        # (sin path); one fused tensor_scalar per plane, fp16 in / f32 out.
        # All six m' land first (they gate the first ACT call).
        MP = pool.tile([_P, _F], f32)
        MS = pool.tile([_P, _F], f32)

        def sl(k):
            return slice(k * _W, (k + 1) * _W)

        # Pool: planes 0,1 from DMA1 then 3,4 from DMA2; DVE: 2 then 5.
        nc.gpsimd.tensor_scalar(MP[:, sl(0)], Q[:, sl(0)], 1.0, None, OP.mod)
        nc.gpsimd.tensor_scalar(MP[:, sl(1)], Q[:, sl(1)], 1.0, None, OP.mod)
        nc.vector.tensor_scalar(MP[:, sl(2)], Q[:, sl(2)], 1.0, None, OP.mod)
        nc.gpsimd.tensor_scalar(MP[:, sl(3)], Q[:, sl(3)], 1.0, None, OP.mod)
        nc.gpsimd.tensor_scalar(MP[:, sl(4)], Q[:, sl(4)], 1.0, None, OP.mod)
        nc.vector.tensor_scalar(MP[:, sl(5)], Q[:, sl(5)], 1.0, None, OP.mod)
        for k, eng in zip(range(_K), ["g", "g", "v", "g", "g", "v"]):
            e = nc.gpsimd if eng == "g" else nc.vector
            e.tensor_scalar(MS[:, sl(k)], Q[:, sl(k)], -0.25, 1.0, OP.add, OP.mod)

        # CF = -cos(2 cq), SF = -sin(2 cq); SF split 5+1 so the P2 Horner
        # chain can start ~500ns before the last sin plane lands
        CF = pool.tile([_P, _F], f32)
        nc.scalar.activation(CF[:], MP[:], ACT.Sin, bias=BNEG[:], scale=SCALE2)
        SF = pool.tile([_P, _F], f32)
        nc.scalar.activation(
            SF[:, 0 : 5 * _W], MS[:, 0 : 5 * _W], ACT.Sin,
            bias=BNEG[:], scale=SCALE2,
        )
        nc.scalar.activation(
            SF[:, 5 * _W : _F], MS[:, 5 * _W : _F], ACT.Sin,
            bias=BNEG[:], scale=SCALE2,
        )

        # A-Horner on Pool: h_k = h_{k-1} * (w_{k-1}/w_k) + CF_k, w = L^2
        HA = [pool.tile([_P, _W], f32, name=f"ha{i}") for i in range(2)]
        nc.gpsimd.scalar_tensor_tensor(
            HA[1][:], CF[:, sl(0)], W2[0] / W2[1], CF[:, sl(1)], OP.mult, OP.add
        )
        for k in range(2, _K):
            nc.gpsimd.scalar_tensor_tensor(
                HA[k % 2][:], HA[(k - 1) % 2][:], W2[k - 1] / W2[k],
                CF[:, sl(k)], OP.mult, OP.add,
            )
        # U = 0.25*(SL2 + A_dev) = h5 * 0.25*w5 + 0.25*SL2   (Pool, right
        # after A5 and before the P2 chain so G1 on DVE lands before G)
        U = pool.tile([_P, _W], f32)
        u_inst = nc.gpsimd.tensor_scalar(
            U[:], HA[(_K - 1) % 2][:], 0.25 * W2[_K - 1], 0.25 * _SL2,
            OP.mult, OP.add,
        )

        # B pairwise tree on DVE: b1 = CF[0:3] + CF[3:6] (312 wide)
        B1 = pool.tile([_P, 3 * _W], f32)
        nc.vector.tensor_add(B1[:], CF[:, 0 : 3 * _W], CF[:, 3 * _W : _F])
        B2 = pool.tile([_P, _W], f32)
        nc.vector.tensor_add(B2[:], B1[:, 0:_W], B1[:, _W : 2 * _W])
        B3 = pool.tile([_P, _W], f32)
        nc.vector.tensor_add(B3[:], B2[:], B1[:, 2 * _W : 3 * _W])
        # t = 1.75 - 0.125*B_dev  (DVE)
        TT = pool.tile([_P, _W], f32)
        nc.vector.tensor_scalar(TT[:], B3[:], -0.125, 1.75, OP.mult, OP.add)
        # G1 = U * t  (DVE)
        G1 = pool.tile([_P, _W], f32)
        nc.vector.tensor_mul(G1[:], U[:], TT[:])

        # P2-Horner on Pool over SF planes (weights L_k); order-only edge
        # keeps the whole A-chain + U ahead of it on Pool so the Sin-gated
        # p1 does not block A5/U at the queue head
        HP = [pool.tile([_P, _W], f32, name=f"hp{i}") for i in range(2)]
        p1_inst = nc.gpsimd.scalar_tensor_tensor(
            HP[1][:], SF[:, sl(0)], L[0] / L[1], SF[:, sl(1)], OP.mult, OP.add
        )
        tile.add_dep_helper(
            p1_inst.ins, u_inst.ins, sync=False, reason="A chain before P2"
        )
        for k in range(2, _K):
            nc.gpsimd.scalar_tensor_tensor(
                HP[k % 2][:], HP[(k - 1) % 2][:], L[k - 1] / L[k],
                SF[:, sl(k)], OP.mult, OP.add,
            )
        # G2 = 0.03125 * P2_true^2 = (0.03125*L5^2 * p5) * p5  (Pool)
        G2 = pool.tile([_P, _W], f32)
        p5 = HP[(_K - 1) % 2]
        nc.gpsimd.scalar_tensor_tensor(
            G2[:], p5[:], 0.03125 * L[_K - 1] * L[_K - 1], p5[:],
            OP.mult, OP.mult,
        )
        # G = G1 - G2 (Pool); fused cost = t / G and row-sum on DVE
        G = pool.tile([_P, _W], f32)
        nc.gpsimd.tensor_sub(G[:], G1[:], G2[:])
        C = pool.tile([_P, _W], f32)
        colsum = pool.tile([_P, 1], f32)
        nc.vector.tensor_tensor_reduce(
            C[:], TT[:], G[:], 1.0, 0.0, OP.divide, OP.add, colsum[:]
        )

        if _SCATTER_OUT:
            # prep runs early (descriptor gen only); the colsum data dep
            # moves to the cheap trigger, skipping the 500ns DGE config and
            # 650ns DGE->DMA delay on the critical tail
            dma_sem = nc.alloc_semaphore("swdge_dma")
            nc.gpsimd.dma_scatter_add(
                out_dram[:, 0:1], colsum[:], OIDX[:], num_idxs=_P,
                num_idxs_reg=_P, elem_size=1, elem_step=64, prepare_only=True,
                sem=dma_sem,
            )
            nc.gpsimd.trigger_dma(count=None)
        else:
            nc.sync.dma_start(out_dram[:], colsum[:])

    nc.compile()
    _CACHE["nc"] = nc
    return nc


def _shard(x):
    # gather the used slice; prefix-sum over joints, scale to pi-units and
    # bias (+4 keeps g positive for mod; +0.25 turns the first Sin into cos)
    qs = np.asarray(x[:, :_T, 6 : 6 + _K], dtype=np.float32)
    g = np.cumsum(qs, axis=-1, dtype=np.float32) * np.float32(1.0 / np.pi)
    g += np.float32(4.25)
    # (ncores, 128, 104, 6) -> planes contiguous along free: (.., 6, 128, 104)
    g = g.reshape(_NCORES, _BPC * _T, _K).transpose(0, 2, 1).reshape(
        _NCORES, _K, _P, _W
    )
    # interleave planes along free axis per partition: (ncores, 128, 624)
    g = np.ascontiguousarray(g.transpose(0, 2, 1, 3)).reshape(
        _NCORES, _P, _F
    )
    return g.astype(np.float16)


def _get_runner():
    """Build the jitted 8-core shard_map executable once (mirrors
    bass2jax.run_bass_via_pjrt's multi-core path) so repeat kernel() calls
    skip retracing/recompiling."""
    if "run" in _CACHE:
        return _CACHE["run"]
    import jax
    from jax.sharding import Mesh, PartitionSpec
    from jax.experimental.shard_map import shard_map
    from concourse import bass2jax

    nc = _get_nc()
    bass2jax.install_neuronx_cc_hook()
    assert nc.dbg_addr is None
    pid_name = nc.partition_id_tensor.name if nc.partition_id_tensor else None
    in_names = ("q", "out") + ((pid_name,) if pid_name else ())

    out_aval = jax.core.ShapedArray((_P, 1), np.float32)

    def _body(q, out_zero):
        operands = [q, out_zero]
        if pid_name is not None:
            operands.append(bass2jax.partition_id_tensor())
        (out,) = bass2jax._bass_exec_p.bind(
            *operands,
            out_avals=(out_aval,),
            in_names=in_names,
            out_names=("out",),
            lowering_input_output_aliases=(),
            sim_require_finite=True,
            sim_require_nnan=True,
            nc=nc,
        )
        return (out,)

    devices = jax.devices()[:_NCORES]
    mesh = Mesh(np.asarray(devices), ("core",))
    sharded = jax.jit(
        shard_map(
            _body,
            mesh=mesh,
            in_specs=(PartitionSpec("core"),) * 2,
            out_specs=(PartitionSpec("core"),),
            check_rep=False,
        ),
        donate_argnums=(1,),
        keep_unused=True,
    )

    def run(planes):
        concat_q = planes.reshape(_NCORES * _P, _F)
        zeros = np.zeros((_NCORES * _P, 1), np.float32)
        (out,) = sharded(concat_q, zeros)
        return np.asarray(out)  # (8*128, 1)

    _CACHE["run"] = run
    return run


def _run_library(planes):
    from concourse.bass_utils import run_bass_kernel_spmd

    res = run_bass_kernel_spmd(
        _get_nc(),
        [{"q": planes[i]} for i in range(_NCORES)],
        list(range(_NCORES)),
    )
    return np.stack([r["out"][:, 0] for r in res.results]).astype(np.float32)


def _run_subprocess(planes):
    """Last resort: the accelerator occasionally reports
    NRT_EXEC_UNIT_UNRECOVERABLE; a fresh process reliably recovers it."""
    import os
    import subprocess
    import sys
    import tempfile

    d = tempfile.mkdtemp()
    inp = os.path.join(d, "planes.npy")
    out = os.path.join(d, "out.npy")
    np.save(inp, planes)
    here = os.path.dirname(os.path.abspath(__file__))
    script = (
        "import sys, numpy as np\n"
        f"sys.path.insert(0, {here!r})\n"
        "import kernel as K\n"
        f"planes = np.load({inp!r})\n"
        "out = K._get_runner()(planes)\n"
        f"np.save({out!r}, out)\n"
    )
    err = None
    for _ in range(2):
        try:
            subprocess.run(
                [sys.executable, "-c", script], check=True, timeout=900,
                stdout=subprocess.DEVNULL, stderr=subprocess.DEVNULL,
            )
            return np.load(out).astype(np.float32)
        except Exception as e:  # retry once; device usually recovers
            err = e
    raise err


def kernel(x, cond, time):
    x = np.asarray(x)
    planes = _shard(x)
    try:
        partials = _get_runner()(planes).astype(np.float32)
    except Exception:
        try:
            # library SPMD runner (covers fast-path/jax API drift)
            partials = _run_library(planes)
        except Exception:
            # fresh process recovers a wedged accelerator
            partials = _run_subprocess(planes)
    return np.float32(partials.sum(dtype=np.float32))


# revision 24
# speedup vs baseline: 2.2044x; 1.3413x over previous
"""Trainium2 Bass kernel for nn_CostFn_18562848653837.

reference(x, cond, time) only reads x[b, j, 6+k] for j in [0,26), k in [0,6)
(~2.6 MB of the 436 MB input; cond/time are unused) and computes, per point,
the reflected mass 1 / (u^T J M^{-1} J^T u) with u = e_x, which reduces via
Sherman-Morrison (M = 2I + 0.5 c c^T, c = cos(cq), s = sin(cq), v = L*s,
cq = cumsum(q)) to

    cost = t / (0.25*(SL2 - A)*t - 0.03125*P2^2),   t = 1.75 + 0.125*B

with the double-angle identities (SL2 = sum L_k^2):

    A  = sum_k L_k^2 cos(2 cq_k)
    B  = sum_k cos(2 cq_k)
    P2 = sum_k L_k sin(2 cq_k)

so only sin/cos of the SINGLE angle 2*cq_k is needed per plane and one
divide, no extra reciprocal.

The host ships AFFINE-prepped planes (same class as the baseline's 1/pi
scale): g_k = cumsum(q)_k / pi + 4.25 as f32. On device, one fused
tensor_scalar per plane produces each trig input:

    m'_k = g_k mod 1          -> Sin(2pi*m' - pi) = -cos(2 cq)   (the +0.25
                                 host offset turns sin into cos)
    m_k  = (g_k - 0.25) mod 1 -> Sin(2pi*m  - pi) = -sin(2 cq)

(mod = np.remainder, result in [0,1); the +4 host bias keeps g > 0 so
floored and truncated mod agree). Both ACT calls share one scale/bias (2pi
shaded one ulp, bias -pi), keeping the Sin table domain satisfied; ACT
order is pinned Cos -> Sin[0:5] -> Sin[5] so the P2 chain starts early.

I/O rides the SWDGE prepare/trigger path on both ends: descriptors are
generated up front (gather for the input, scatter-add for the output, with
an identity-iota int16 index tile built on device), and the cheap
trigger_dma carries the data dependency - the input lands without paying
the DGE-config + DGE->DMA-delay + sem-prop chain a dma_start pays, so the
~1.3us ACT Sin-table load (hoisted to t~0 by a dep-free warm-up Sin)
becomes the head critical path, and the output trigger fires right after
the final reduce.

A/P2 are Horner chains of scalar_tensor_tensor ops (weight ratios folded
into the chain, tail weights folded into downstream fused scalars); B is a
pairwise tree. Work spread: mods + A + U + P2 + G2 + G on Pool, mods +
B-tree + t + G1 + fused divide-and-reduce (tensor_tensor_reduce) on DVE,
the Sins on ACT.

Sharding: pure data parallel over batch - core i gets batches
[512*i, 512*(i+1)), i.e. 512*26 = 13312 points as a (128, 104) tile per
plane, planes contiguous along free, rows padded to 640 f32 (gather
elem_size must be a 256B multiple). Each core emits one f32 (128,1)
partial-sum column scattered into a (128, 64)-padded out; host adds.
"""

import numpy as np

_P, _W, _K = 128, 104, 6
_F = _K * _W
_FP = 640  # row padded to a 256-byte multiple for dma_gather
_NCORES = 8
_B, _H, _T = 4096, 1024, 26
_BPC = _B // _NCORES  # batches per core

_CACHE = {}

_LV = np.arange(1, 7, dtype=np.float64) * 0.1 + 0.3  # [0.4 .. 0.9]
_SL2 = float((_LV * _LV).sum())  # 2.71


def _get_nc():
    if "nc" in _CACHE:
        return _CACHE["nc"]

    import concourse.tile as tile
    import concourse.mybir as mybir
    import concourse.bass_isa as bass_isa
    from concourse import bacc

    PI32 = float(np.float32(np.pi))
    # One-ulp-shaded 2*pi: m in [0,1) so SCALE2*m - pi stays strictly inside
    # the Sin table domain [-pi, pi].
    SCALE2 = float(np.float32(2.0 * np.pi * (1.0 - 2.0**-23)))
    L = [float(v) for v in _LV]
    W2 = [float(v * v) for v in _LV]

    f32 = mybir.dt.float32
    i16 = mybir.dt.int16
    OP = mybir.AluOpType
    ACT = mybir.ActivationFunctionType

    nc = bacc.Bacc(
        "TRN2", target_bir_lowering=False, debug=False, num_devices=_NCORES,
        disable_frame_to_traceback=True, num_swdge_queues=2,
    )
    q_dram = nc.dram_tensor("q", [_P, _FP], f32, kind="ExternalInput")
    # scatter-add needs a 256B-multiple dram row stride -> pad to 64 f32
    out_dram = nc.dram_tensor("out", [_P, 64], f32, kind="ExternalOutput")

    with (
        tile.TileContext(nc) as tc,
        tc.tile_pool(name="pool", bufs=1) as pool,
    ):
        # constant -pi bias tile for both ACT Sins
        BNEG = pool.tile([_P, 1], f32)
        nc.vector.memset(BNEG[:], -PI32)

        # Dep-free dummy Sin on the pre-initialized const-1.0 AP: hoists the
        # ~1.3us Sin table-set load to t~0 so it runs behind the input path.
        one_ap = nc.const_aps.aps[(f32, 1.0)]
        WARM = pool.tile([_P, 1], f32)
        nc.scalar.activation(WARM[:], one_ap[:_P], ACT.Sin)

        # identity gather/scatter indices, wrapped in 16 partitions:
        # value at [p, s] = p + 16*s  (only partitions 0..15 are read)
        OIDX = pool.tile([_P, 8], i16)
        nc.vector.memset(OIDX[:], 0)  # rows 16+ unused, keep in-bounds
        nc.gpsimd.iota(OIDX[0:16, :], pattern=[[16, 8]], base=0,
                       channel_multiplier=1)

        # input: one 128-row gather via the SWDGE prepare/trigger path
        Q = pool.tile([_P, _FP], f32)
        in_sem = nc.alloc_semaphore("swdge_in")
        nc.gpsimd.dma_gather(
            Q[:].rearrange("p (one f) -> p one f", one=1), q_dram[:],
            OIDX[:], num_idxs=_P, num_idxs_reg=_P,
            elem_size=_FP, prepare_only=True, sem=in_sem, queue_num=0,
        )
        in_trig = nc.gpsimd.trigger_dma(count=None, queue_num=0)

        # range reduction: m' = g mod 1 (cos path), m = (g - 0.25) mod 1
        # (sin path); one fused tensor_scalar per plane. All six m' land
        # first (they gate the Cos call).
        MP = pool.tile([_P, _F], f32)
        MS = pool.tile([_P, _F], f32)

        def sl(k):
            return slice(k * _W, (k + 1) * _W)

        for k, eng in zip(range(_K), ["g", "g", "v", "g", "g", "v"]):
            e = nc.gpsimd if eng == "g" else nc.vector
            # gather writes Q at trigger time; readers wait the DMA sem
            e.tensor_scalar(
                MP[:, sl(k)], Q[:, sl(k)], 1.0, None, OP.mod
            )._wait_ge(in_sem, 16)
        for k, eng in zip(range(_K), ["g", "g", "v", "g", "g", "v"]):
            e = nc.gpsimd if eng == "g" else nc.vector
            e.tensor_scalar(
                MS[:, sl(k)], Q[:, sl(k)], -0.25, 1.0, OP.add, OP.mod
            )._wait_ge(in_sem, 16)

        # CF = -cos(2 cq), SF = -sin(2 cq); SF split 5+1 so the P2 Horner
        # starts ~500ns before the last sin plane lands. ACT order pinned.
        CF = pool.tile([_P, _F], f32)
        cos_i = nc.scalar.activation(
            CF[:], MP[:], ACT.Sin, bias=BNEG[:], scale=SCALE2
        )
        SF = pool.tile([_P, _F], f32)
        sin1_i = nc.scalar.activation(
            SF[:, 0 : 5 * _W], MS[:, 0 : 5 * _W], ACT.Sin,
            bias=BNEG[:], scale=SCALE2,
        )
        sin2_i = nc.scalar.activation(
            SF[:, 5 * _W : _F], MS[:, 5 * _W : _F], ACT.Sin,
            bias=BNEG[:], scale=SCALE2,
        )
        tile.add_dep_helper(sin1_i.ins, cos_i.ins, sync=False,
                            reason="cos before sin")
        tile.add_dep_helper(sin2_i.ins, sin1_i.ins, sync=False,
                            reason="sin order")

        # A-Horner on Pool: h_k = h_{k-1} * (w_{k-1}/w_k) + CF_k, w = L^2
        HA = [pool.tile([_P, _W], f32, name=f"ha{i}") for i in range(2)]
        nc.gpsimd.scalar_tensor_tensor(
            HA[1][:], CF[:, sl(0)], W2[0] / W2[1], CF[:, sl(1)], OP.mult, OP.add
        )
        for k in range(2, _K):
            nc.gpsimd.scalar_tensor_tensor(
                HA[k % 2][:], HA[(k - 1) % 2][:], W2[k - 1] / W2[k],
                CF[:, sl(k)], OP.mult, OP.add,
            )
        # U = 0.25*(SL2 + A_dev) = h5 * 0.25*w5 + 0.25*SL2  (Pool, before P2
        # so G1 on DVE lands before G needs it)
        U = pool.tile([_P, _W], f32)
        u_inst = nc.gpsimd.tensor_scalar(
            U[:], HA[(_K - 1) % 2][:], 0.25 * W2[_K - 1], 0.25 * _SL2,
            OP.mult, OP.add,
        )

        # B pairwise tree + t + G1 on DVE
        B1 = pool.tile([_P, 3 * _W], f32)
        nc.vector.tensor_add(B1[:], CF[:, 0 : 3 * _W], CF[:, 3 * _W : _F])
        B2 = pool.tile([_P, _W], f32)
        nc.vector.tensor_add(B2[:], B1[:, 0:_W], B1[:, _W : 2 * _W])
        B3 = pool.tile([_P, _W], f32)
        nc.vector.tensor_add(B3[:], B2[:], B1[:, 2 * _W : 3 * _W])
        # t = 1.75 - 0.125*B_dev
        TT = pool.tile([_P, _W], f32)
        nc.vector.tensor_scalar(TT[:], B3[:], -0.125, 1.75, OP.mult, OP.add)
        # G1 = U * t
        G1 = pool.tile([_P, _W], f32)
        nc.vector.tensor_mul(G1[:], U[:], TT[:])

        # P2-Horner on Pool (weights L_k); order-only edge keeps A+U ahead
        # of the Sin-gated p1 at the Pool queue head
        HP = [pool.tile([_P, _W], f32, name=f"hp{i}") for i in range(2)]
        p1_inst = nc.gpsimd.scalar_tensor_tensor(
            HP[1][:], SF[:, sl(0)], L[0] / L[1], SF[:, sl(1)], OP.mult, OP.add
        )
        tile.add_dep_helper(p1_inst.ins, u_inst.ins, sync=False,
                            reason="A chain before P2")
        for k in range(2, _K):
            nc.gpsimd.scalar_tensor_tensor(
                HP[k % 2][:], HP[(k - 1) % 2][:], L[k - 1] / L[k],
                SF[:, sl(k)], OP.mult, OP.add,
            )
        # G2 = 0.03125 * P2_true^2 = (0.03125*L5^2 * p5) * p5  (Pool)
        G2 = pool.tile([_P, _W], f32)
        p5 = HP[(_K - 1) % 2]
        nc.gpsimd.scalar_tensor_tensor(
            G2[:], p5[:], 0.03125 * L[_K - 1] * L[_K - 1], p5[:],
            OP.mult, OP.mult,
        )
        # G = G1 - G2 (Pool); fused cost = t / G and row-sum on DVE
        G = pool.tile([_P, _W], f32)
        nc.gpsimd.tensor_sub(G[:], G1[:], G2[:])
        C = pool.tile([_P, _W], f32)
        colsum = pool.tile([_P, 1], f32)
        nc.vector.tensor_tensor_reduce(
            C[:], TT[:], G[:], 1.0, 0.0, OP.divide, OP.add, colsum[:]
        )

        # output: scatter-add prep early, cheap trigger after the reduce
        out_sem = nc.alloc_semaphore("swdge_out")
        nc.gpsimd.dma_scatter_add(
            out_dram[:, 0:1], colsum[:], OIDX[:], num_idxs=_P,
            num_idxs_reg=_P, elem_size=1, elem_step=64, prepare_only=True,
            sem=out_sem, queue_num=1,
        )
        nc.gpsimd.trigger_dma(count=None, queue_num=1)

    nc.compile()
    _CACHE["nc"] = nc
    return nc


def _shard(x):
    # gather the used slice; prefix-sum over joints, scale to pi-units and
    # bias (+4 keeps g positive for mod; +0.25 turns the first Sin into cos)
    qs = np.asarray(x[:, :_T, 6 : 6 + _K], dtype=np.float32)
    g = np.cumsum(qs, axis=-1, dtype=np.float32) * np.float32(1.0 / np.pi)
    g += np.float32(4.25)
    # (ncores, 6, 128, 104) planes, then plane-contiguous rows padded to 640
    g = g.reshape(_NCORES, _BPC * _T, _K).transpose(0, 2, 1).reshape(
        _NCORES, _K, _P, _W
    )
    planes = np.zeros((_NCORES, _P, _FP), dtype=np.float32)
    planes[:, :, :_F] = g.transpose(0, 2, 1, 3).reshape(_NCORES, _P, _F)
    return planes


def _get_runner():
    """Build the jitted 8-core shard_map executable once (mirrors
    bass2jax.run_bass_via_pjrt's multi-core path) so repeat kernel() calls
    skip retracing/recompiling."""
    if "run" in _CACHE:
        return _CACHE["run"]
    import jax
    from jax.sharding import Mesh, PartitionSpec
    from jax.experimental.shard_map import shard_map
    from concourse import bass2jax

    nc = _get_nc()
    bass2jax.install_neuronx_cc_hook()
    assert nc.dbg_addr is None
    pid_name = nc.partition_id_tensor.name if nc.partition_id_tensor else None
    in_names = ("q", "out") + ((pid_name,) if pid_name else ())

    out_aval = jax.core.ShapedArray((_P, 64), np.float32)

    def _body(q, out_zero):
        operands = [q, out_zero]
        if pid_name is not None:
            operands.append(bass2jax.partition_id_tensor())
        (out,) = bass2jax._bass_exec_p.bind(
            *operands,
            out_avals=(out_aval,),
            in_names=in_names,
            out_names=("out",),
            lowering_input_output_aliases=(),
            sim_require_finite=True,
            sim_require_nnan=True,
            nc=nc,
        )
        return (out,)

    devices = jax.devices()[:_NCORES]
    mesh = Mesh(np.asarray(devices), ("core",))
    sharded = jax.jit(
        shard_map(
            _body,
            mesh=mesh,
            in_specs=(PartitionSpec("core"),) * 2,
            out_specs=(PartitionSpec("core"),),
            check_rep=False,
        ),
        donate_argnums=(1,),
        keep_unused=True,
    )

    def run(planes):
        concat_q = planes.reshape(_NCORES * _P, _FP)
        zeros = np.zeros((_NCORES * _P, 64), np.float32)
        (out,) = sharded(concat_q, zeros)
        return np.asarray(out)[:, 0]  # (8*128,)

    _CACHE["run"] = run
    return run


def _run_library(planes):
    from concourse.bass_utils import run_bass_kernel_spmd

    res = run_bass_kernel_spmd(
        _get_nc(),
        [{"q": planes[i], "out": np.zeros((_P, 64), np.float32)}
         for i in range(_NCORES)],
        list(range(_NCORES)),
    )
    return np.stack([r["out"][:, 0] for r in res.results]).astype(np.float32)


def _run_subprocess(planes):
    """Last resort: the accelerator occasionally reports
    NRT_EXEC_UNIT_UNRECOVERABLE; a fresh process reliably recovers it."""
    import os
    import subprocess
    import sys
    import tempfile

    d = tempfile.mkdtemp()
    inp = os.path.join(d, "planes.npy")
    out = os.path.join(d, "out.npy")
    np.save(inp, planes)
    here = os.path.dirname(os.path.abspath(__file__))
    script = (
        "import sys, numpy as np\n"
        f"sys.path.insert(0, {here!r})\n"
        "import kernel as K\n"
        f"planes = np.load({inp!r})\n"
        "out = K._get_runner()(planes)\n"
        f"np.save({out!r}, out)\n"
    )
    err = None
    for _ in range(2):
        try:
            subprocess.run(
                [sys.executable, "-c", script], check=True, timeout=900,
                stdout=subprocess.DEVNULL, stderr=subprocess.DEVNULL,
            )
            return np.load(out).astype(np.float32)
        except Exception as e:  # retry once; device usually recovers
            err = e
    raise err


def kernel(x, cond, time):
    x = np.asarray(x)
    planes = _shard(x)
    try:
        partials = _get_runner()(planes).astype(np.float32)
    except Exception:
        try:
            # library SPMD runner (covers fast-path/jax API drift)
            partials = _run_library(planes)
        except Exception:
            # fresh process recovers a wedged accelerator
            partials = _run_subprocess(planes)
    return np.float32(partials.sum(dtype=np.float32))


# revision 28
# speedup vs baseline: 2.2202x; 1.0072x over previous
"""Trainium2 Bass kernel for nn_CostFn_18562848653837.

reference(x, cond, time) only reads x[b, j, 6+k] for j in [0,26), k in [0,6)
(~2.6 MB of the 436 MB input; cond/time are unused) and computes, per point,
the reflected mass 1 / (u^T J M^{-1} J^T u) with u = e_x, which reduces via
Sherman-Morrison (M = 2I + 0.5 c c^T, c = cos(cq), s = sin(cq), v = L*s,
cq = cumsum(q)) to

    cost = t / (0.25*(SL2 - A)*t - 0.03125*P2^2),   t = 1.75 + 0.125*B

with the double-angle identities (SL2 = sum L_k^2):

    A  = sum_k L_k^2 cos(2 cq_k)
    B  = sum_k cos(2 cq_k)
    P2 = sum_k L_k sin(2 cq_k)

so only sin/cos of the SINGLE angle 2*cq_k is needed per plane and one
divide, no extra reciprocal.

The host ships AFFINE-prepped planes (same class as the baseline's 1/pi
scale): g_k = cumsum(q)_k / pi + 4.25 as f32. On device, one fused
tensor_scalar per plane produces each trig input:

    m'_k = g_k mod 1          -> Sin(2pi*m' - pi) = -cos(2 cq)   (the +0.25
                                 host offset turns sin into cos)
    m_k  = (g_k - 0.25) mod 1 -> Sin(2pi*m  - pi) = -sin(2 cq)

(mod = np.remainder, result in [0,1); the +4 host bias keeps g > 0 so
floored and truncated mod agree). Both ACT calls share one scale/bias (2pi
shaded one ulp, bias -pi), keeping the Sin table domain satisfied; ACT
order is pinned Cos -> Sin[0:5] -> Sin[5] so the P2 chain starts early.

I/O rides the SWDGE prepare/trigger path on both ends: descriptors are
generated up front (gather for the input, scatter-add for the output, with
an identity-iota int16 index tile built on device), and the cheap
trigger_dma carries the data dependency - the input lands without paying
the DGE-config + DGE->DMA-delay + sem-prop chain a dma_start pays, so the
~1.3us ACT Sin-table load (hoisted to t~0 by a dep-free warm-up Sin)
becomes the head critical path, and the output trigger fires right after
the final reduce.

A/P2 are Horner chains of scalar_tensor_tensor ops (weight ratios folded
into the chain, tail weights folded into downstream fused scalars); B is a
pairwise tree. Work spread: mods + A + U + P2 + G2 + G on Pool, mods +
B-tree + t + G1 + fused divide-and-reduce (tensor_tensor_reduce) on DVE,
the Sins on ACT.

Sharding: pure data parallel over batch - core i gets batches
[512*i, 512*(i+1)), i.e. 512*26 = 13312 points as a (128, 104) tile per
plane, planes contiguous along free, rows padded to 640 f32 (gather
elem_size must be a 256B multiple). Each core emits one f32 (128,1)
partial-sum column scattered into a (128, 64)-padded out; host adds.
"""

import numpy as np

_P, _W, _K = 128, 104, 6
_F = _K * _W
_FP = 640  # row padded to a 256-byte multiple for dma_gather
_NCORES = 8
_B, _H, _T = 4096, 1024, 26
_BPC = _B // _NCORES  # batches per core

_CACHE = {}

_LV = np.arange(1, 7, dtype=np.float64) * 0.1 + 0.3  # [0.4 .. 0.9]
_SL2 = float((_LV * _LV).sum())  # 2.71


def _get_nc():
    if "nc" in _CACHE:
        return _CACHE["nc"]

    import concourse.tile as tile
    import concourse.mybir as mybir
    import concourse.bass_isa as bass_isa
    from concourse import bacc

    PI32 = float(np.float32(np.pi))
    # One-ulp-shaded 2*pi: m in [0,1) so SCALE2*m - pi stays strictly inside
    # the Sin table domain [-pi, pi].
    SCALE2 = float(np.float32(2.0 * np.pi * (1.0 - 2.0**-23)))
    L = [float(v) for v in _LV]
    W2 = [float(v * v) for v in _LV]

    f32 = mybir.dt.float32
    i16 = mybir.dt.int16
    OP = mybir.AluOpType
    ACT = mybir.ActivationFunctionType

    nc = bacc.Bacc(
        "TRN2", target_bir_lowering=False, debug=False, num_devices=_NCORES,
        disable_frame_to_traceback=True, num_swdge_queues=2,
    )
    q_dram = nc.dram_tensor("q", [_P, _FP], f32, kind="ExternalInput")
    # scatter-add needs a 256B-multiple dram row stride -> pad to 64 f32
    out_dram = nc.dram_tensor("out", [_P, 64], f32, kind="ExternalOutput")

    with (
        tile.TileContext(nc) as tc,
        tc.tile_pool(name="pool", bufs=1) as pool,
    ):
        # constant -pi bias tile for both ACT Sins
        BNEG = pool.tile([_P, 1], f32)
        nc.vector.memset(BNEG[:], -PI32)

        # Dep-free dummy Sin on the pre-initialized const-1.0 AP: hoists the
        # ~1.3us Sin table-set load to t~0 so it runs behind the input path.
        one_ap = nc.const_aps.aps[(f32, 1.0)]
        WARM = pool.tile([_P, 1], f32)
        nc.scalar.activation(WARM[:], one_ap[:_P], ACT.Sin)

        # identity gather/scatter indices, wrapped in 16 partitions:
        # value at [p, s] = p + 16*s  (only partitions 0..15 are read)
        OIDX = pool.tile([_P, 8], i16)
        nc.vector.memset(OIDX[:], 0)  # rows 16+ unused, keep in-bounds
        nc.gpsimd.iota(OIDX[0:16, :], pattern=[[16, 8]], base=0,
                       channel_multiplier=1)
        # scatter-reduce indices: position i = t*16+q maps cost element
        # (i%128, i//128) -> out row i%128 = 16*(t%8)+q; the scatter-add DMA
        # performs the final 13-way row reduction (128*13 idxs, ~80 descs
        # fits the 128-entry SWDGE ring)
        OIDX2 = pool.tile([_P, _W], i16)
        nc.vector.memset(OIDX2[:], 0)
        nc.gpsimd.iota(OIDX2[0:16, :], pattern=[[0, 13], [16, 8]], base=0,
                       channel_multiplier=1)

        # input: one 128-row gather via the SWDGE prepare/trigger path
        Q = pool.tile([_P, _FP], f32)
        in_sem = nc.alloc_semaphore("swdge_in")
        nc.gpsimd.dma_gather(
            Q[:].rearrange("p (one f) -> p one f", one=1), q_dram[:],
            OIDX[:], num_idxs=_P, num_idxs_reg=_P,
            elem_size=_FP, prepare_only=True, sem=in_sem, queue_num=0,
        )
        in_trig = nc.gpsimd.trigger_dma(count=None, queue_num=0)

        # range reduction: m' = g mod 1 (cos path), m = (g - 0.25) mod 1
        # (sin path); one fused tensor_scalar per plane. All six m' land
        # first (they gate the Cos call).
        MP = pool.tile([_P, _F], f32)
        MS = pool.tile([_P, _F], f32)

        def sl(k):
            return slice(k * _W, (k + 1) * _W)

        for k, eng in zip(range(_K), ["g", "g", "v", "g", "g", "v"]):
            e = nc.gpsimd if eng == "g" else nc.vector
            # gather writes Q at trigger time; readers wait the DMA sem
            e.tensor_scalar(
                MP[:, sl(k)], Q[:, sl(k)], 1.0, None, OP.mod
            )._wait_ge(in_sem, 16)
        for k, eng in zip(range(_K), ["g", "g", "v", "g", "g", "v"]):
            e = nc.gpsimd if eng == "g" else nc.vector
            e.tensor_scalar(
                MS[:, sl(k)], Q[:, sl(k)], -0.25, 1.0, OP.add, OP.mod
            )._wait_ge(in_sem, 16)

        # CF = -cos(2 cq), SF = -sin(2 cq); SF split 5+1 so the P2 Horner
        # starts ~500ns before the last sin plane lands. ACT order pinned.
        CF = pool.tile([_P, _F], f32)
        cos_i = nc.scalar.activation(
            CF[:], MP[:], ACT.Sin, bias=BNEG[:], scale=SCALE2
        )
        SF = pool.tile([_P, _F], f32)
        sin1_i = nc.scalar.activation(
            SF[:, 0 : 5 * _W], MS[:, 0 : 5 * _W], ACT.Sin,
            bias=BNEG[:], scale=SCALE2,
        )
        sin2_i = nc.scalar.activation(
            SF[:, 5 * _W : _F], MS[:, 5 * _W : _F], ACT.Sin,
            bias=BNEG[:], scale=SCALE2,
        )
        tile.add_dep_helper(sin1_i.ins, cos_i.ins, sync=False,
                            reason="cos before sin")
        tile.add_dep_helper(sin2_i.ins, sin1_i.ins, sync=False,
                            reason="sin order")

        # A-Horner on Pool: h_k = h_{k-1} * (w_{k-1}/w_k) + CF_k, w = L^2
        HA = [pool.tile([_P, _W], f32, name=f"ha{i}") for i in range(2)]
        nc.gpsimd.scalar_tensor_tensor(
            HA[1][:], CF[:, sl(0)], W2[0] / W2[1], CF[:, sl(1)], OP.mult, OP.add
        )
        for k in range(2, _K):
            nc.gpsimd.scalar_tensor_tensor(
                HA[k % 2][:], HA[(k - 1) % 2][:], W2[k - 1] / W2[k],
                CF[:, sl(k)], OP.mult, OP.add,
            )
        # U = 0.25*(SL2 + A_dev) = h5 * 0.25*w5 + 0.25*SL2  (Pool, before P2
        # so G1 on DVE lands before G needs it)
        U = pool.tile([_P, _W], f32)
        u_inst = nc.gpsimd.tensor_scalar(
            U[:], HA[(_K - 1) % 2][:], 0.25 * W2[_K - 1], 0.25 * _SL2,
            OP.mult, OP.add,
        )

        # B pairwise tree + t + G1 on DVE
        B1 = pool.tile([_P, 3 * _W], f32)
        nc.vector.tensor_add(B1[:], CF[:, 0 : 3 * _W], CF[:, 3 * _W : _F])
        B2 = pool.tile([_P, _W], f32)
        nc.vector.tensor_add(B2[:], B1[:, 0:_W], B1[:, _W : 2 * _W])
        B3 = pool.tile([_P, _W], f32)
        nc.vector.tensor_add(B3[:], B2[:], B1[:, 2 * _W : 3 * _W])
        # t = 1.75 - 0.125*B_dev
        TT = pool.tile([_P, _W], f32)
        nc.vector.tensor_scalar(TT[:], B3[:], -0.125, 1.75, OP.mult, OP.add)
        # G1 = U * t
        G1 = pool.tile([_P, _W], f32)
        nc.vector.tensor_mul(G1[:], U[:], TT[:])

        # P2-Horner on Pool (weights L_k); order-only edge keeps A+U ahead
        # of the Sin-gated p1 at the Pool queue head
        HP = [pool.tile([_P, _W], f32, name=f"hp{i}") for i in range(2)]
        p1_inst = nc.gpsimd.scalar_tensor_tensor(
            HP[1][:], SF[:, sl(0)], L[0] / L[1], SF[:, sl(1)], OP.mult, OP.add
        )
        tile.add_dep_helper(p1_inst.ins, u_inst.ins, sync=False,
                            reason="A chain before P2")
        for k in range(2, _K):
            nc.gpsimd.scalar_tensor_tensor(
                HP[k % 2][:], HP[(k - 1) % 2][:], L[k - 1] / L[k],
                SF[:, sl(k)], OP.mult, OP.add,
            )
        # G2 = 0.03125 * P2_true^2 = (0.03125*L5^2 * p5) * p5  (Pool)
        G2 = pool.tile([_P, _W], f32)
        p5 = HP[(_K - 1) % 2]
        nc.gpsimd.scalar_tensor_tensor(
            G2[:], p5[:], 0.03125 * L[_K - 1] * L[_K - 1], p5[:],
            OP.mult, OP.mult,
        )
        # G = G1 - G2, cost C = t / G, pairwise-fold 104 -> 13 columns (all
        # Pool, same engine as the trigger: no cross-engine sem on the tail)
        G = pool.tile([_P, _W], f32)
        nc.gpsimd.tensor_sub(G[:], G1[:], G2[:])
        C = pool.tile([_P, _W], f32)
        nc.gpsimd.tensor_tensor(C[:], TT[:], G[:], OP.divide)
        nc.gpsimd.tensor_add(C[:, 0:52], C[:, 0:52], C[:, 52:104])
        nc.gpsimd.tensor_add(C[:, 0:26], C[:, 0:26], C[:, 26:52])
        nc.gpsimd.tensor_add(C[:, 0:13], C[:, 0:13], C[:, 13:26])

        # output: the scatter-add DMA reduces the 13 folded cost columns of
        # each partition row into out[row, 0]; prep early, trigger after
        out_sem = nc.alloc_semaphore("swdge_out")
        nc.gpsimd.dma_scatter_add(
            out_dram[:, 0:1],
            C[:, 0:13].rearrange("p (s one) -> p s one", one=1),
            OIDX2[:], num_idxs=_P * 13,
            num_idxs_reg=_P * 13, elem_size=1, elem_step=64,
            prepare_only=True, sem=out_sem, queue_num=1,
        )
        nc.gpsimd.trigger_dma(count=None, queue_num=1)

    nc.compile()
    _CACHE["nc"] = nc
    return nc


def _shard(x):
    # gather the used slice; prefix-sum over joints, scale to pi-units and
    # bias (+4 keeps g positive for mod; +0.25 turns the first Sin into cos)
    qs = np.asarray(x[:, :_T, 6 : 6 + _K], dtype=np.float32)
    g = np.cumsum(qs, axis=-1, dtype=np.float32) * np.float32(1.0 / np.pi)
    g += np.float32(4.25)
    # (ncores, 6, 128, 104) planes, then plane-contiguous rows padded to 640
    g = g.reshape(_NCORES, _BPC * _T, _K).transpose(0, 2, 1).reshape(
        _NCORES, _K, _P, _W
    )
    planes = np.zeros((_NCORES, _P, _FP), dtype=np.float32)
    planes[:, :, :_F] = g.transpose(0, 2, 1, 3).reshape(_NCORES, _P, _F)
    return planes


def _get_runner():
    """Build the jitted 8-core shard_map executable once (mirrors
    bass2jax.run_bass_via_pjrt's multi-core path) so repeat kernel() calls
    skip retracing/recompiling."""
    if "run" in _CACHE:
        return _CACHE["run"]
    import jax
    from jax.sharding import Mesh, PartitionSpec
    from jax.experimental.shard_map import shard_map
    from concourse import bass2jax

    nc = _get_nc()
    bass2jax.install_neuronx_cc_hook()
    assert nc.dbg_addr is None
    pid_name = nc.partition_id_tensor.name if nc.partition_id_tensor else None
    in_names = ("q", "out") + ((pid_name,) if pid_name else ())

    out_aval = jax.core.ShapedArray((_P, 64), np.float32)

    def _body(q, out_zero):
        operands = [q, out_zero]
        if pid_name is not None:
            operands.append(bass2jax.partition_id_tensor())
        (out,) = bass2jax._bass_exec_p.bind(
            *operands,
            out_avals=(out_aval,),
            in_names=in_names,
            out_names=("out",),
            lowering_input_output_aliases=(),
            sim_require_finite=True,
            sim_require_nnan=True,
            nc=nc,
        )
        return (out,)

    devices = jax.devices()[:_NCORES]
    mesh = Mesh(np.asarray(devices), ("core",))
    sharded = jax.jit(
        shard_map(
            _body,
            mesh=mesh,
            in_specs=(PartitionSpec("core"),) * 2,
            out_specs=(PartitionSpec("core"),),
            check_rep=False,
        ),
        donate_argnums=(1,),
        keep_unused=True,
    )

    def run(planes):
        concat_q = planes.reshape(_NCORES * _P, _FP)
        zeros = np.zeros((_NCORES * _P, 64), np.float32)
        (out,) = sharded(concat_q, zeros)
        return np.asarray(out)[:, 0]  # (8*128,)

    _CACHE["run"] = run
    return run


def _run_library(planes):
    from concourse.bass_utils import run_bass_kernel_spmd

    res = run_bass_kernel_spmd(
        _get_nc(),
        [{"q": planes[i], "out": np.zeros((_P, 64), np.float32)}
         for i in range(_NCORES)],
        list(range(_NCORES)),
    )
    return np.stack([r["out"][:, 0] for r in res.results]).astype(np.float32)


def _run_subprocess(planes):
    """Last resort: the accelerator occasionally reports
    NRT_EXEC_UNIT_UNRECOVERABLE; a fresh process reliably recovers it."""
    import os
    import subprocess
    import sys
    import tempfile

    d = tempfile.mkdtemp()
    inp = os.path.join(d, "planes.npy")
    out = os.path.join(d, "out.npy")
    np.save(inp, planes)
    here = os.path.dirname(os.path.abspath(__file__))
    script = (
        "import sys, numpy as np\n"
        f"sys.path.insert(0, {here!r})\n"
        "import kernel as K\n"
        f"planes = np.load({inp!r})\n"
        "out = K._get_runner()(planes)\n"
        f"np.save({out!r}, out)\n"
    )
    err = None
    for _ in range(2):
        try:
            subprocess.run(
                [sys.executable, "-c", script], check=True, timeout=900,
                stdout=subprocess.DEVNULL, stderr=subprocess.DEVNULL,
            )
            return np.load(out).astype(np.float32)
        except Exception as e:  # retry once; device usually recovers
            err = e
    raise err


def kernel(x, cond, time):
    x = np.asarray(x)
    planes = _shard(x)
    try:
        partials = _get_runner()(planes).astype(np.float32)
    except Exception:
        try:
            # library SPMD runner (covers fast-path/jax API drift)
            partials = _run_library(planes)
        except Exception:
            # fresh process recovers a wedged accelerator
            partials = _run_subprocess(planes)
    return np.float32(partials.sum(dtype=np.float32))
